# revision 49
# baseline (speedup 1.0000x reference)
"""Multi-head attention (B=4, S=2048, D=1024, H=16, causal+pad mask) on 8 TRN2 cores.

Sharding: core c handles batch b=c//2 and head-group g=c%2 (8 heads, 512 model
dims of the q/k/v projections).  Each core computes q/k/v projections for its
head slice, causal attention, and a partial output projection; the host sums
the two partial outputs per batch and adds bo.

v2 schedule: single continuous PE instruction stream.  Attention score/AV
tiles are pipelined per head-pair with depth-2 lag for the softmax (exp on the
scalar engine); projection chunks for the NEXT attention block and output
projections of the PREVIOUS block are interleaved as filler matmuls, budgeted
by emitted-cycle fractions, so the PE never idles and stays at max p-state.

Softmax details:
  - scoresT[k, q] computed per 128-k-tile; causal mask inside the diagonal
    128x128 block applied by ACCUMULATING an upper-triangle -1e30 constant
    into PSUM via an identity matmul (no vector-engine masking).
  - exp fused with the padding-mask bias and the 1/sqrt(Dh) scale.
  - row sums come from a ones column appended to v; normalization divides
    av PSUM by the partition-broadcast sums on the vector engine.
  - v bias is added with a rank-1 (contract=1) ones x bvrow matmul into the
    v projection PSUM, so o/s + bv falls out of the normalization for free.

PSUM budget (8 banks): scores 2x[128,1024] (4) + av 2x[65,512] (2) +
shared proj/outproj pool 2x[128,512] (2).
"""

import numpy as np
from collections import deque

B, S, D, H, Dh = 4, 2048, 1024, 16, 64
NCORES = 8
SC1 = 512          # s-chunk == attention q-chunk
NJ1 = S // SC1     # 4
NKT = S // 128     # 16
NPR = 4            # head-pair tiles per core (8 heads)
DEPTH = 3          # sc->av pipeline lag (in k-tiles)

_CACHE = {}


def _build_nc():
    import concourse.bacc as bacc
    import concourse.mybir as mybir
    import concourse.tile as tile
    from contextlib import ExitStack

    F32 = mybir.dt.float32
    BF16 = mybir.dt.bfloat16
    ExpF = mybir.ActivationFunctionType.Exp

    nc = bacc.Bacc("TRN2", target_bir_lowering=False, debug=False,
                   num_devices=NCORES)

    xw_d = nc.declare_dram_parameter("xw", [NJ1, 128, 8 * SC1], BF16, isOutput=False)
    wq_d = nc.declare_dram_parameter("wq", [128, 4096], BF16, isOutput=False)
    wk_d = nc.declare_dram_parameter("wk", [128, 4096], BF16, isOutput=False)
    wv_d = nc.declare_dram_parameter("wv", [128, 4096], BF16, isOutput=False)
    wo_d = nc.declare_dram_parameter("wo", [128, 4096], BF16, isOutput=False)
    bq_d = nc.declare_dram_parameter("bq2", [128, 4], F32, isOutput=False)
    bk_d = nc.declare_dram_parameter("bk2", [128, 4], F32, isOutput=False)
    kb_d = nc.declare_dram_parameter("kbias", [128, NKT], F32, isOutput=False)
    tn_d = nc.declare_dram_parameter("trimask2", [128, 256], BF16, isOutput=False)
    bv_d = nc.declare_dram_parameter("bvrow", [1, 512], BF16, isOutput=False)
    out_d = nc.declare_dram_parameter("out", [S, D], F32, isOutput=True)

    with tile.TileContext(nc) as tc, ExitStack() as ctx:
        cpool = ctx.enter_context(tc.tile_pool(name="consts", bufs=1))
        bigpool = ctx.enter_context(tc.tile_pool(name="big", bufs=1))
        wpool = ctx.enter_context(tc.tile_pool(name="wp", bufs=1))
        xpool = ctx.enter_context(tc.tile_pool(name="xp", bufs=4))
        qpool = ctx.enter_context(tc.tile_pool(name="qp", bufs=8))
        ppool = ctx.enter_context(tc.tile_pool(name="pp", bufs=8))
        opool = ctx.enter_context(tc.tile_pool(name="op", bufs=12))
        spool = ctx.enter_context(tc.tile_pool(name="sp", bufs=4))
        bpool = ctx.enter_context(tc.tile_pool(name="bp", bufs=4))
        rpool = ctx.enter_context(tc.tile_pool(name="rp", bufs=6))
        scpool = ctx.enter_context(tc.tile_pool(name="sc", bufs=2, space="PSUM"))
        avpool = ctx.enter_context(tc.tile_pool(name="av", bufs=2, space="PSUM"))
        mmpool = ctx.enter_context(tc.tile_pool(name="mm", bufs=2, space="PSUM"))

        # ---- input DMAs: x0/wq interleaved in 4 column-chunks so the first
        # projection matmuls can start after ~1/4 of each transfer ----
        xt0 = xpool.tile([128, 8 * SC1], BF16, name="xt0", tag="x")
        wq_t = wpool.tile([128, 4096], BF16, name="wq_t")
        bq_t = cpool.tile([128, 4], F32, name="bq_t")
        bk_t = cpool.tile([128, 4], F32, name="bk_t")
        wk_t = wpool.tile([128, 4096], BF16, name="wk_t")
        wv_t = wpool.tile([128, 4096], BF16, name="wv_t")
        for ch in range(2):
            cs = slice(ch * 2048, (ch + 1) * 2048)
            nc.sync.dma_start(wq_t[:, cs], wq_d[:, cs])
            nc.sync.dma_start(xt0[:, cs], xw_d[0, :, cs])
            if ch == 0:
                nc.sync.dma_start(bq_t[:], bq_d[:])
                nc.sync.dma_start(bk_t[:], bk_d[:])
        for ch in range(2):
            cs = slice(ch * 2048, (ch + 1) * 2048)
            nc.sync.dma_start(wk_t[:, cs], wk_d[:, cs])
        nc.sync.dma_start(wv_t[:], wv_d[:])
        bv_t = cpool.tile([1, 512], BF16, name="bv_t")
        nc.sync.dma_start(bv_t[:], bv_d[:])
        kb_t = cpool.tile([128, NKT], F32, name="kb_t")
        nc.sync.dma_start(kb_t[:], kb_d[:])
        tn_t = cpool.tile([128, 256], BF16, name="tn_t")
        nc.sync.dma_start(tn_t[:], tn_d[:])

        XT = {0: xt0}
        for j in (1, 2, 3):
            xt = xpool.tile([128, 8 * SC1], BF16, name=f"xt{j}", tag="x")
            nc.sync.dma_start(xt[:], xw_d[j])
            XT[j] = xt
        wo_t = wpool.tile([128, 4096], BF16, name="wo_t")
        nc.sync.dma_start(wo_t[:], wo_d[:])

        # K (transposed, pair-stacked) and v (+ones col per head) persist.
        K_t = bigpool.tile([128, NPR * S], BF16, name="K_t")
        vb_t = bigpool.tile([128, NKT * 520], BF16, name="vb_t")
        bvf_t = bigpool.tile([128, 512], BF16, name="bvf_t")
        nc.gpsimd.partition_broadcast(bvf_t[:], bv_t[:], channels=128)

        QT = {}
        OT = {}

        # ---- emission helpers ----
        def q_group(pr, j, pool=None):
            # qq = [[q_a, 0], [0, q_b]] so score matmuls can use the full
            # 128-partition K tile as lhsT (contract-64 matmuls run ~1.5x
            # slower on HW); the zero block kills the other head's term.
            xt = XT[j]
            qt = qpool.tile([128, 1024], BF16, name=f"q{pr}_{j}", tag="q")
            ps = (pool or mmpool).tile([128, 512], F32, name=f"qps{pr}_{j}",
                                       tag="mm" if pool is None else "sc")
            for ci in range(8):
                nc.tensor.matmul(
                    ps[:],
                    wq_t[:, ci * 512 + pr * 128: ci * 512 + pr * 128 + 128],
                    xt[:, ci * SC1: (ci + 1) * SC1],
                    start=(ci == 0), stop=(ci == 7))
            nc.gpsimd.memset(qt[64:128, 0:512], 0.0)
            nc.gpsimd.memset(qt[0:64, 512:1024], 0.0)
            nc.vector.tensor_scalar_add(
                qt[0:64, 0:512], ps[0:64, :], bq_t[0:64, pr: pr + 1])
            nc.vector.tensor_scalar_add(
                qt[64:128, 512:1024], ps[64:128, :], bq_t[64:128, pr: pr + 1])
            QT[(pr, j)] = qt

        def k_group(pr, j, pool=None):
            xt = XT[j]
            ps = (pool or mmpool).tile([128, 512], F32, name=f"kps{pr}_{j}",
                                       tag="mm" if pool is None else "sc")
            for ci in range(8):
                nc.tensor.matmul(
                    ps[:],
                    wk_t[:, ci * 512 + pr * 128: ci * 512 + pr * 128 + 128],
                    xt[:, ci * SC1: (ci + 1) * SC1],
                    start=(ci == 0), stop=(ci == 7))
            nc.vector.tensor_scalar_add(
                K_t[:, pr * S + j * SC1: pr * S + (j + 1) * SC1], ps[:],
                bk_t[:, pr: pr + 1])

        def v_group(st, j, pool=None):
            xt = XT[j]
            kt = 4 * j + st
            ps = (pool or mmpool).tile([128, 512], F32, name=f"vps{st}_{j}",
                                       tag="mm" if pool is None else "sc")
            for ci in range(8):
                nc.tensor.matmul(
                    ps[:],
                    xt[:, ci * SC1 + st * 128: ci * SC1 + st * 128 + 128],
                    wv_t[:, ci * 512: (ci + 1) * 512],
                    start=(ci == 0), stop=(ci == 7))
            vslot = vb_t[:, kt * 520: (kt + 1) * 520]
            nc.vector.tensor_add(
                vslot.rearrange("p (h e) -> p h e", h=8)[:, :, 0:64],
                ps[:].rearrange("p (h e) -> p h e", h=8),
                bvf_t[:].rearrange("p (h e) -> p h e", h=8))
            nc.gpsimd.memset(
                vslot.rearrange("p (h e) -> p h e", h=8)[:, :, 64:65], 1.0)

        def out_group(J, si, dm, tail=False):
            ps = mmpool.tile([128, 512], F32, name=f"ops{si}_{dm}", tag="mm")
            for pr in range(NPR):
                nc.tensor.matmul(
                    ps[:],
                    OT[(pr, J)][:, (si - 4 * J) * 128: (si - 4 * J) * 128 + 128],
                    wo_t[:, pr * 1024 + dm * 512: pr * 1024 + (dm + 1) * 512],
                    start=(pr == 0), stop=(pr == 3))
            res = rpool.tile([128, 512], F32, name=f"res{si}_{dm}", tag="res")
            if tail:
                nc.scalar.copy(res[:], ps[:])
            else:
                nc.vector.tensor_copy(res[:], ps[:])
            nc.sync.dma_start(
                out_d[si * 128: (si + 1) * 128, dm * 512: (dm + 1) * 512],
                res[:])

        def emit_sc(pr, J, kt, qt):
            r = kt - 4 * J
            off = 128 * r if r >= 0 else 0
            sc = scpool.tile([128, 1024], F32, name=f"sc{pr}_{J}_{kt}",
                             tag="sc")
            Ktile = K_t[:, pr * S + kt * 128: pr * S + kt * 128 + 128]
            for h in (0, 1):
                base = h * 512
                nc.tensor.matmul(
                    sc[:, base + off: base + 512], Ktile,
                    qt[:, base + off: base + 512],
                    start=True, stop=True)
            P = ppool.tile([128, 1024], BF16, name=f"P{pr}_{J}_{kt}", tag="p")
            nc.scalar.activation(
                P[:].rearrange("p (h q) -> p h q", h=2)[:, :, off:512],
                sc[:].rearrange("p (h q) -> p h q", h=2)[:, :, off:512],
                ExpF, bias=kb_t[:, kt: kt + 1], scale=0.125)
            if r >= 0:
                both = P[:].rearrange("p (h q) -> p h q", h=2)[:, :, off: off + 128]
                nc.vector.tensor_mul(
                    both, both,
                    tn_t[:].rearrange("p (h q) -> p h q", h=2))
            return P, off

        def emit_av(pr, av_a, av_b, kt, P, off, nkt):
            nc.tensor.matmul(
                av_a[:, off:512],
                vb_t[:, kt * 520 + (2 * pr) * 65: kt * 520 + (2 * pr) * 65 + 65],
                P[:, off:512],
                start=(kt == 0), stop=(kt == nkt - 1))
            nc.tensor.matmul(
                av_b[:, off:512],
                vb_t[:, kt * 520 + (2 * pr + 1) * 65: kt * 520 + (2 * pr + 1) * 65 + 65],
                P[:, 512 + off:1024],
                start=(kt == 0), stop=(kt == nkt - 1))

        def norm_pr(pr, J, av_a, av_b):
            # 1/rowsum straight from the av PSUM ones-row, broadcast across
            # partitions on gpsimd, then one multiply per head half.
            # custom-DVE recip can't read PSUM on HW: gather both ones-rows
            # into one SBUF tile, then a single recip over [1, 1024].
            sr = spool.tile([1, 1024], F32, name=f"sr{pr}_{J}", tag="s")
            nc.vector.tensor_copy(sr[:, 0:512], av_a[64:65, :])
            nc.vector.tensor_copy(sr[:, 512:1024], av_b[64:65, :])
            rr = spool.tile([1, 1024], F32, name=f"rr{pr}_{J}", tag="s")
            nc.vector.reciprocal_approx_fast(rr[:], sr[:])
            rra = rr[:, 0:512]
            rrb = rr[:, 512:1024]
            sba = bpool.tile([64, 512], F32, name=f"sba{pr}_{J}", tag="sb")
            nc.gpsimd.partition_broadcast(sba[:], rra, channels=64)
            sbb = bpool.tile([64, 512], F32, name=f"sbb{pr}_{J}", tag="sb")
            nc.gpsimd.partition_broadcast(sbb[:], rrb, channels=64)
            ot = opool.tile([128, 512], BF16, name=f"o{pr}_{J}", tag="o")
            nc.vector.tensor_mul(ot[0:64, :], av_a[0:64, :], sba[:])
            nc.vector.tensor_mul(ot[64:128, :], av_b[0:64, :], sbb[:])
            OT[(pr, J)] = ot

        # ---- filler budgeting ----
        projQ = deque()
        outQ = deque()
        st8 = {"proj_done": 0, "proj_total": 0, "out_done": 0, "out_total": 0,
               "attn_done": 0, "attn_total": 1, "out_dl": 0.90,
               "out_ok": True, "proj_dl": 0.80}

        def push_proj(cost, fn):
            projQ.append((cost, fn))
            st8["proj_total"] += cost

        def push_out(cost, args):
            outQ.append((cost, args))
            st8["out_total"] += cost

        def fill():
            frac = st8["attn_done"] / st8["attn_total"]
            while projQ and st8["proj_done"] < st8["proj_total"] * min(
                    1.0, frac / st8["proj_dl"]):
                cost, fn = projQ.popleft()
                fn()
                st8["proj_done"] += cost
            while st8["out_ok"] and outQ and st8["out_done"] < st8[
                    "out_total"] * min(1.0, frac / st8["out_dl"]):
                cost, (jj, s, d) = outQ.popleft()
                out_group(jj, s, d)
                st8["out_done"] += cost

        def drain_all(tail=False):
            while projQ:
                cost, fn = projQ.popleft()
                fn()
            while outQ:
                cost, (jj, s, d) = outQ.popleft()
                out_group(jj, s, d, tail=tail)

        # ---- PE warm-up on garbage SBUF (no deps; ramps the p-state
        # during the DMA/runtime-init dead window) ----
        warm = bigpool.tile([128, 512], BF16, name="warm")
        nc.vector.memset(warm[:], 0.0)
        wps = mmpool.tile([128, 512], F32, name="wps", tag="mm")
        for i in range(32):
            nc.tensor.matmul(wps[:], warm[:, 0:128], warm[:],
                             start=(i == 0), stop=(i == 31))

        # ---- chunk 0 projections (PE ramp-up; borrow the idle score pool
        # for extra PSUM buffering) ----
        for pr in range(NPR):
            q_group(pr, 0, pool=(scpool if pr % 2 else None))
            k_group(pr, 0, pool=(scpool if pr % 2 == 0 else None))
        for st in range(4):
            v_group(st, 0, pool=(scpool if st % 2 else None))

        # ---- main attention loop ----
        for J in range(4):
            nkt = 4 * (J + 1)
            # reset budgets for this J
            st8["attn_done"] = 0
            st8["attn_total"] = 32768 * J + 20480
            st8["proj_done"] = 0
            st8["proj_total"] = 0
            st8["out_done"] = 0
            st8["out_total"] = 0
            st8["proj_dl"] = 0.80
            if J < 3:
                jn = J + 1
                for pr in range(NPR):
                    push_proj(4096, (lambda p=pr, j=jn: q_group(p, j)))
                if J < 2:
                    for pr in range(NPR):
                        push_proj(4096, (lambda p=pr, j=jn: k_group(p, j)))
                    for st in range(4):
                        push_proj(4096, (lambda s=st, j=jn: v_group(s, j)))
            if J == 3:
                # k/v chunk-3 fillers land here (needed ~20% into J=3),
                # relieving the exp-bound tail block
                st8["proj_dl"] = 0.22
                for x in range(4):
                    push_proj(4096, (lambda s=x: v_group(s, 3)))
                    push_proj(4096, (lambda p=x: k_group(p, 3)))
            # outproj fillers are delayed by one extra J so they land in J=3,
            # which has no proj fillers left and is otherwise exp-throughput
            # limited; at J=3 the deadline is pushed past 1.0 so a few groups
            # spill into the drain after the last norm chain.
            st8["out_dl"] = 2.00 if J == 3 else 0.90
            st8["out_ok"] = True
            for Jp in ([] if J < 2 else ([0] if J == 2 else [1, 2])):
                for si in range(4 * Jp, 4 * Jp + 4):
                    for dm in range(2):
                        push_out(2048, (Jp, si, dm))

            for pr in range(NPR):
                if J == 3 and pr == 3:
                    st8["out_ok"] = False
                av_a = avpool.tile([65, 512], F32, name=f"ava{pr}_{J}",
                                   tag="av")
                av_b = avpool.tile([65, 512], F32, name=f"avb{pr}_{J}",
                                   tag="av")
                qt = QT[(pr, J)]
                pending = []
                for kt in range(nkt):
                    P, off = emit_sc(pr, J, kt, qt)
                    pending.append((kt, P, off))
                    r = kt - 4 * J
                    st8["attn_done"] += 4 * (512 - (128 * r if r >= 0 else 0))
                    fill()
                    if len(pending) > DEPTH:
                        pkt, pP, poff = pending.pop(0)
                        emit_av(pr, av_a, av_b, pkt, pP, poff, nkt)
                while pending:
                    pkt, pP, poff = pending.pop(0)
                    emit_av(pr, av_a, av_b, pkt, pP, poff, nkt)
                    if projQ:
                        cost, fn = projQ.popleft()
                        fn()
                        st8["proj_done"] += cost
                    fill()
                if J == 3 and pr == 3:
                    drain_all(tail=True)
                norm_pr(pr, J, av_a, av_b)
                fill()

        # ---- tail: final output projection ----
        drain_all()
        for si in range(12, 16):
            for dm in range(2):
                out_group(3, si, dm, tail=True)

    nc.compile()
    return nc


def _get_nc():
    if "nc" not in _CACHE:
        _CACHE["nc"] = _build_nc()
    return _CACHE["nc"]


def make_in_maps(x, mask, Wq, bq, Wk, bk, Wv, bv, Wo, bo):
    import ml_dtypes
    f32 = np.float32
    bf16 = ml_dtypes.bfloat16
    # trineg[kc, qc] = -1e30 where qc < kc (query before key -> masked)
    tri = np.triu(np.ones((128, 128), f32)).astype(bf16)
    trimask2 = np.concatenate([tri, tri], axis=1)
    in_maps = []
    for c in range(NCORES):
        b, g = c // 2, c % 2
        xb = np.asarray(x[b], f32)  # [S, D]
        xw = np.ascontiguousarray(
            xb.reshape(NJ1, SC1, 8, 128).transpose(0, 3, 2, 1).reshape(
                NJ1, 128, 8 * SC1)).astype(bf16)
        sl = slice(g * 512, (g + 1) * 512)

        def wlay(W):  # [512,1024] rows=outputs -> [128, 8*512]
            return np.ascontiguousarray(
                np.asarray(W[sl], f32).reshape(512, 8, 128).transpose(2, 1, 0)
                .reshape(128, 4096)).astype(bf16)

        wo = np.ascontiguousarray(
            np.asarray(Wo[:, sl], f32).T.reshape(4, 128, 1024)
            .transpose(1, 0, 2).reshape(128, 4096)).astype(bf16)
        bq2 = np.ascontiguousarray(np.asarray(bq[sl], f32).reshape(4, 128).T)
        bk2 = np.ascontiguousarray(np.asarray(bk[sl], f32).reshape(4, 128).T)
        bvrow = np.asarray(bv[sl], f32).reshape(1, 512).astype(bf16)
        kbias = np.ascontiguousarray(
            np.where(np.asarray(mask[b]) == 0, f32(-1e30), f32(0.0))
            .astype(f32).reshape(NKT, 128).T)
        in_maps.append({
            "xw": xw, "wq": wlay(Wq), "wk": wlay(Wk), "wv": wlay(Wv),
            "wo": wo, "bq2": bq2, "bk2": bk2, "bvrow": bvrow,
            "kbias": kbias, "trimask2": trimask2,
        })
    return in_maps


def kernel(x, mask, Wq, bq, Wk, bk, Wv, bv, Wo, bo):
    from concourse.bass_utils import run_bass_kernel_spmd

    nc = _get_nc()
    in_maps = make_in_maps(x, mask, Wq, bq, Wk, bk, Wv, bv, Wo, bo)
    res = run_bass_kernel_spmd(nc, in_maps, list(range(NCORES))).results
    out = np.empty((B, S, D), np.float32)
    bo32 = np.asarray(bo, np.float32)
    for b in range(B):
        out[b] = res[2 * b]["out"] + res[2 * b + 1]["out"] + bo32
    return out


# revision 51
# speedup vs baseline: 1.1887x; 1.1887x over previous
"""Multi-head attention (B=4, S=2048, D=1024, H=16, causal+pad mask) on 8 TRN2 cores.

Sharding: core c handles batch b=c//2 and head-group g=c%2 (8 heads, 512 model
dims of the q/k/v projections).  Each core computes q/k/v projections for its
head slice, causal attention, and a partial output projection; the host sums
the two partial outputs per batch and adds bo.

v2 schedule: single continuous PE instruction stream.  Attention score/AV
tiles are pipelined per head-pair with depth-2 lag for the softmax (exp on the
scalar engine); projection chunks for the NEXT attention block and output
projections of the PREVIOUS block are interleaved as filler matmuls, budgeted
by emitted-cycle fractions, so the PE never idles and stays at max p-state.

Softmax details:
  - scoresT[k, q] computed per 128-k-tile; causal mask inside the diagonal
    128x128 block applied by ACCUMULATING an upper-triangle -1e30 constant
    into PSUM via an identity matmul (no vector-engine masking).
  - exp fused with the padding-mask bias and the 1/sqrt(Dh) scale.
  - row sums come from a ones column appended to v; normalization divides
    av PSUM by the partition-broadcast sums on the vector engine.
  - v bias is added with a rank-1 (contract=1) ones x bvrow matmul into the
    v projection PSUM, so o/s + bv falls out of the normalization for free.

PSUM budget (8 banks): scores 2x[128,1024] (4) + av 2x[65,512] (2) +
shared proj/outproj pool 2x[128,512] (2).
"""

import numpy as np
from collections import deque

B, S, D, H, Dh = 4, 2048, 1024, 16, 64
NCORES = 8
SC1 = 512          # s-chunk == attention q-chunk
NJ1 = S // SC1     # 4
NKT = S // 128     # 16
NPR = 4            # head-pair tiles per core (8 heads)
DEPTH = 3          # sc->av pipeline lag (in k-tiles)

_CACHE = {}


def _build_nc():
    import concourse.bacc as bacc
    import concourse.mybir as mybir
    import concourse.tile as tile
    from contextlib import ExitStack

    F32 = mybir.dt.float32
    BF16 = mybir.dt.bfloat16
    ExpF = mybir.ActivationFunctionType.Exp

    nc = bacc.Bacc("TRN2", target_bir_lowering=False, debug=False,
                   num_devices=NCORES)

    xw_d = nc.declare_dram_parameter("xw", [NJ1, 128, 8 * SC1], BF16, isOutput=False)
    wq_d = nc.declare_dram_parameter("wq", [128, 4096], BF16, isOutput=False)
    wk_d = nc.declare_dram_parameter("wk", [128, 4096], BF16, isOutput=False)
    wv_d = nc.declare_dram_parameter("wv", [128, 4096], BF16, isOutput=False)
    wo_d = nc.declare_dram_parameter("wo", [128, 4096], BF16, isOutput=False)
    bq_d = nc.declare_dram_parameter("bq2", [128, 4], F32, isOutput=False)
    bk_d = nc.declare_dram_parameter("bk2", [128, 4], F32, isOutput=False)
    kb_d = nc.declare_dram_parameter("kbias", [128, NKT], F32, isOutput=False)
    tn_d = nc.declare_dram_parameter("trimask2", [128, 256], BF16, isOutput=False)
    bv_d = nc.declare_dram_parameter("bvrow", [1, 512], BF16, isOutput=False)
    out_d = nc.declare_dram_parameter("out", [S, D], F32, isOutput=True)

    with tile.TileContext(nc) as tc, ExitStack() as ctx:
        cpool = ctx.enter_context(tc.tile_pool(name="consts", bufs=1))
        bigpool = ctx.enter_context(tc.tile_pool(name="big", bufs=1))
        wpool = ctx.enter_context(tc.tile_pool(name="wp", bufs=1))
        xpool = ctx.enter_context(tc.tile_pool(name="xp", bufs=4))
        qpool = ctx.enter_context(tc.tile_pool(name="qp", bufs=8))
        ppool = ctx.enter_context(tc.tile_pool(name="pp", bufs=8))
        opool = ctx.enter_context(tc.tile_pool(name="op", bufs=12))
        spool = ctx.enter_context(tc.tile_pool(name="sp", bufs=4))
        bpool = ctx.enter_context(tc.tile_pool(name="bp", bufs=4))
        rpool = ctx.enter_context(tc.tile_pool(name="rp", bufs=6))
        scpool = ctx.enter_context(tc.tile_pool(name="sc", bufs=2, space="PSUM"))
        avpool = ctx.enter_context(tc.tile_pool(name="av", bufs=2, space="PSUM"))
        mmpool = ctx.enter_context(tc.tile_pool(name="mm", bufs=2, space="PSUM"))

        # ---- input DMAs: x0/wq interleaved in 4 column-chunks so the first
        # projection matmuls can start after ~1/4 of each transfer ----
        xt0 = xpool.tile([128, 8 * SC1], BF16, name="xt0", tag="x")
        wq_t = wpool.tile([128, 4096], BF16, name="wq_t")
        bq_t = cpool.tile([128, 4], F32, name="bq_t")
        bk_t = cpool.tile([128, 4], F32, name="bk_t")
        wk_t = wpool.tile([128, 4096], BF16, name="wk_t")
        wv_t = wpool.tile([128, 4096], BF16, name="wv_t")
        for ch in range(2):
            cs = slice(ch * 2048, (ch + 1) * 2048)
            nc.sync.dma_start(wq_t[:, cs], wq_d[:, cs])
            nc.sync.dma_start(xt0[:, cs], xw_d[0, :, cs])
            if ch == 0:
                nc.sync.dma_start(bq_t[:], bq_d[:])
                nc.sync.dma_start(bk_t[:], bk_d[:])
        for ch in range(2):
            cs = slice(ch * 2048, (ch + 1) * 2048)
            nc.sync.dma_start(wk_t[:, cs], wk_d[:, cs])
        nc.sync.dma_start(wv_t[:], wv_d[:])
        bv_t = cpool.tile([1, 512], BF16, name="bv_t")
        nc.sync.dma_start(bv_t[:], bv_d[:])
        kb_t = cpool.tile([128, NKT], F32, name="kb_t")
        nc.sync.dma_start(kb_t[:], kb_d[:])
        tn_t = cpool.tile([128, 256], BF16, name="tn_t")
        nc.sync.dma_start(tn_t[:], tn_d[:])

        XT = {0: xt0}
        for j in (1, 2, 3):
            xt = xpool.tile([128, 8 * SC1], BF16, name=f"xt{j}", tag="x")
            nc.sync.dma_start(xt[:], xw_d[j])
            XT[j] = xt
        wo_t = wpool.tile([128, 4096], BF16, name="wo_t")
        nc.sync.dma_start(wo_t[:], wo_d[:])

        # K (transposed, pair-stacked) and v (+ones col per head) persist.
        K_t = bigpool.tile([128, NPR * S], BF16, name="K_t")
        vb_t = bigpool.tile([128, NKT * 520], BF16, name="vb_t")
        bvf_t = bigpool.tile([128, 512], BF16, name="bvf_t")
        nc.gpsimd.partition_broadcast(bvf_t[:], bv_t[:], channels=128)

        QT = {}
        OT = {}

        # ---- emission helpers ----
        def q_group(pr, j, pool=None):
            # qq = [[q_a, 0], [0, q_b]] so score matmuls can use the full
            # 128-partition K tile as lhsT (contract-64 matmuls run ~1.5x
            # slower on HW); the zero block kills the other head's term.
            xt = XT[j]
            qt = qpool.tile([128, 1024], BF16, name=f"q{pr}_{j}", tag="q")
            ps = (pool or mmpool).tile([128, 512], F32, name=f"qps{pr}_{j}",
                                       tag="mm" if pool is None else "sc")
            for ci in range(8):
                nc.tensor.matmul(
                    ps[:],
                    wq_t[:, ci * 512 + pr * 128: ci * 512 + pr * 128 + 128],
                    xt[:, ci * SC1: (ci + 1) * SC1],
                    start=(ci == 0), stop=(ci == 7))
            nc.gpsimd.memset(qt[64:128, 0:512], 0.0)
            nc.gpsimd.memset(qt[0:64, 512:1024], 0.0)
            nc.vector.tensor_scalar_add(
                qt[0:64, 0:512], ps[0:64, :], bq_t[0:64, pr: pr + 1])
            nc.vector.tensor_scalar_add(
                qt[64:128, 512:1024], ps[64:128, :], bq_t[64:128, pr: pr + 1])
            QT[(pr, j)] = qt

        def k_group(pr, j, pool=None):
            xt = XT[j]
            ps = (pool or mmpool).tile([128, 512], F32, name=f"kps{pr}_{j}",
                                       tag="mm" if pool is None else "sc")
            for ci in range(8):
                nc.tensor.matmul(
                    ps[:],
                    wk_t[:, ci * 512 + pr * 128: ci * 512 + pr * 128 + 128],
                    xt[:, ci * SC1: (ci + 1) * SC1],
                    start=(ci == 0), stop=(ci == 7))
            nc.vector.tensor_scalar_add(
                K_t[:, pr * S + j * SC1: pr * S + (j + 1) * SC1], ps[:],
                bk_t[:, pr: pr + 1])

        def v_group(st, j, pool=None):
            xt = XT[j]
            kt = 4 * j + st
            ps = (pool or mmpool).tile([128, 512], F32, name=f"vps{st}_{j}",
                                       tag="mm" if pool is None else "sc")
            for ci in range(8):
                nc.tensor.matmul(
                    ps[:],
                    xt[:, ci * SC1 + st * 128: ci * SC1 + st * 128 + 128],
                    wv_t[:, ci * 512: (ci + 1) * 512],
                    start=(ci == 0), stop=(ci == 7))
            vslot = vb_t[:, kt * 520: (kt + 1) * 520]
            nc.vector.tensor_add(
                vslot.rearrange("p (h e) -> p h e", h=8)[:, :, 0:64],
                ps[:].rearrange("p (h e) -> p h e", h=8),
                bvf_t[:].rearrange("p (h e) -> p h e", h=8))
            nc.gpsimd.memset(
                vslot.rearrange("p (h e) -> p h e", h=8)[:, :, 64:65], 1.0)

        def out_group(J, si, dm, tail=False):
            ps = mmpool.tile([128, 512], F32, name=f"ops{si}_{dm}", tag="mm")
            for pr in range(NPR):
                nc.tensor.matmul(
                    ps[:],
                    OT[(pr, J)][:, (si - 4 * J) * 128: (si - 4 * J) * 128 + 128],
                    wo_t[:, pr * 1024 + dm * 512: pr * 1024 + (dm + 1) * 512],
                    start=(pr == 0), stop=(pr == 3))
            res = rpool.tile([128, 512], F32, name=f"res{si}_{dm}", tag="res")
            if tail:
                nc.scalar.copy(res[:], ps[:])
            else:
                nc.vector.tensor_copy(res[:], ps[:])
            nc.sync.dma_start(
                out_d[si * 128: (si + 1) * 128, dm * 512: (dm + 1) * 512],
                res[:])

        def emit_sc(pr, J, kt, qt):
            r = kt - 4 * J
            off = 128 * r if r >= 0 else 0
            sc = scpool.tile([128, 1024], F32, name=f"sc{pr}_{J}_{kt}",
                             tag="sc")
            Ktile = K_t[:, pr * S + kt * 128: pr * S + kt * 128 + 128]
            for h in (0, 1):
                base = h * 512
                nc.tensor.matmul(
                    sc[:, base + off: base + 512], Ktile,
                    qt[:, base + off: base + 512],
                    start=True, stop=True)
            P = ppool.tile([128, 1024], BF16, name=f"P{pr}_{J}_{kt}", tag="p")
            nc.scalar.activation(
                P[:].rearrange("p (h q) -> p h q", h=2)[:, :, off:512],
                sc[:].rearrange("p (h q) -> p h q", h=2)[:, :, off:512],
                ExpF, bias=kb_t[:, kt: kt + 1], scale=0.125)
            if r >= 0:
                both = P[:].rearrange("p (h q) -> p h q", h=2)[:, :, off: off + 128]
                nc.vector.tensor_mul(
                    both, both,
                    tn_t[:].rearrange("p (h q) -> p h q", h=2))
            return P, off

        def emit_av(pr, av_a, av_b, kt, P, off, nkt):
            nc.tensor.matmul(
                av_a[:, off:512],
                vb_t[:, kt * 520 + (2 * pr) * 65: kt * 520 + (2 * pr) * 65 + 65],
                P[:, off:512],
                start=(kt == 0), stop=(kt == nkt - 1))
            nc.tensor.matmul(
                av_b[:, off:512],
                vb_t[:, kt * 520 + (2 * pr + 1) * 65: kt * 520 + (2 * pr + 1) * 65 + 65],
                P[:, 512 + off:1024],
                start=(kt == 0), stop=(kt == nkt - 1))

        def norm_pr(pr, J, av_a, av_b):
            # 1/rowsum straight from the av PSUM ones-row, broadcast across
            # partitions on gpsimd, then one multiply per head half.
            # custom-DVE recip can't read PSUM on HW: gather both ones-rows
            # into one SBUF tile, then a single recip over [1, 1024].
            sr = spool.tile([1, 1024], F32, name=f"sr{pr}_{J}", tag="s")
            nc.vector.tensor_copy(sr[:, 0:512], av_a[64:65, :])
            nc.vector.tensor_copy(sr[:, 512:1024], av_b[64:65, :])
            rr = spool.tile([1, 1024], F32, name=f"rr{pr}_{J}", tag="s")
            nc.vector.reciprocal_approx_fast(rr[:], sr[:])
            rra = rr[:, 0:512]
            rrb = rr[:, 512:1024]
            sba = bpool.tile([64, 512], F32, name=f"sba{pr}_{J}", tag="sb")
            nc.gpsimd.partition_broadcast(sba[:], rra, channels=64)
            sbb = bpool.tile([64, 512], F32, name=f"sbb{pr}_{J}", tag="sb")
            nc.gpsimd.partition_broadcast(sbb[:], rrb, channels=64)
            ot = opool.tile([128, 512], BF16, name=f"o{pr}_{J}", tag="o")
            nc.vector.tensor_mul(ot[0:64, :], av_a[0:64, :], sba[:])
            nc.vector.tensor_mul(ot[64:128, :], av_b[0:64, :], sbb[:])
            OT[(pr, J)] = ot

        # ---- filler budgeting ----
        projQ = deque()
        outQ = deque()
        st8 = {"proj_done": 0, "proj_total": 0, "out_done": 0, "out_total": 0,
               "attn_done": 0, "attn_total": 1, "out_dl": 0.90,
               "out_ok": True, "proj_dl": 0.80}

        def push_proj(cost, fn):
            projQ.append((cost, fn))
            st8["proj_total"] += cost

        def push_out(cost, args):
            outQ.append((cost, args))
            st8["out_total"] += cost

        def fill():
            frac = st8["attn_done"] / st8["attn_total"]
            while projQ and st8["proj_done"] < st8["proj_total"] * min(
                    1.0, frac / st8["proj_dl"]):
                cost, fn = projQ.popleft()
                fn()
                st8["proj_done"] += cost
            while st8["out_ok"] and outQ and st8["out_done"] < st8[
                    "out_total"] * min(1.0, frac / st8["out_dl"]):
                cost, (jj, s, d) = outQ.popleft()
                out_group(jj, s, d)
                st8["out_done"] += cost

        def drain_all(tail=False):
            while projQ:
                cost, fn = projQ.popleft()
                fn()
            while outQ:
                cost, (jj, s, d) = outQ.popleft()
                out_group(jj, s, d, tail=tail)

        # ---- PE warm-up on garbage SBUF (no deps; ramps the p-state
        # during the DMA/runtime-init dead window) ----
        warm = bigpool.tile([128, 512], BF16, name="warm")
        nc.vector.memset(warm[:], 0.0)
        wps = mmpool.tile([128, 512], F32, name="wps", tag="mm")
        for i in range(32):
            nc.tensor.matmul(wps[:], warm[:, 0:128], warm[:],
                             start=(i == 0), stop=(i == 31))

        # ---- chunk 0 projections (PE ramp-up; borrow the idle score pool
        # for extra PSUM buffering) ----
        for pr in range(NPR):
            q_group(pr, 0, pool=(scpool if pr % 2 else None))
            k_group(pr, 0, pool=(scpool if pr % 2 == 0 else None))
        for st in range(4):
            v_group(st, 0, pool=(scpool if st % 2 else None))

        # ---- main attention loop ----
        for J in range(4):
            nkt = 4 * (J + 1)
            # reset budgets for this J
            st8["attn_done"] = 0
            st8["attn_total"] = 32768 * J + 20480
            st8["proj_done"] = 0
            st8["proj_total"] = 0
            st8["out_done"] = 0
            st8["out_total"] = 0
            st8["proj_dl"] = 0.80
            if J < 3:
                jn = J + 1
                for pr in range(NPR):
                    push_proj(4096, (lambda p=pr, j=jn: q_group(p, j)))
                if J < 2:
                    for pr in range(NPR):
                        push_proj(4096, (lambda p=pr, j=jn: k_group(p, j)))
                    for st in range(4):
                        push_proj(4096, (lambda s=st, j=jn: v_group(s, j)))
            if J == 3:
                # k/v chunk-3 fillers land here (needed ~20% into J=3),
                # relieving the exp-bound tail block
                st8["proj_dl"] = 0.22
                for x in range(4):
                    push_proj(4096, (lambda s=x: v_group(s, 3)))
                    push_proj(4096, (lambda p=x: k_group(p, 3)))
            # outproj fillers are delayed by one extra J so they land in J=3,
            # which has no proj fillers left and is otherwise exp-throughput
            # limited; at J=3 the deadline is pushed past 1.0 so a few groups
            # spill into the drain after the last norm chain.
            st8["out_dl"] = 2.00 if J == 3 else 0.90
            st8["out_ok"] = True
            for Jp in ([] if J < 2 else ([0] if J == 2 else [1, 2])):
                for si in range(4 * Jp, 4 * Jp + 4):
                    for dm in range(2):
                        push_out(2048, (Jp, si, dm))

            for pr in range(NPR):
                if J == 3 and pr == 3:
                    st8["out_ok"] = False
                av_a = avpool.tile([65, 512], F32, name=f"ava{pr}_{J}",
                                   tag="av")
                av_b = avpool.tile([65, 512], F32, name=f"avb{pr}_{J}",
                                   tag="av")
                qt = QT[(pr, J)]
                pending = []
                for kt in range(nkt):
                    P, off = emit_sc(pr, J, kt, qt)
                    pending.append((kt, P, off))
                    r = kt - 4 * J
                    st8["attn_done"] += 4 * (512 - (128 * r if r >= 0 else 0))
                    fill()
                    if len(pending) > DEPTH:
                        pkt, pP, poff = pending.pop(0)
                        emit_av(pr, av_a, av_b, pkt, pP, poff, nkt)
                while pending:
                    pkt, pP, poff = pending.pop(0)
                    emit_av(pr, av_a, av_b, pkt, pP, poff, nkt)
                    if projQ:
                        cost, fn = projQ.popleft()
                        fn()
                        st8["proj_done"] += cost
                    fill()
                if J == 3 and pr == 3:
                    drain_all(tail=True)
                norm_pr(pr, J, av_a, av_b)
                fill()

        # ---- tail: final output projection ----
        drain_all()
        for si in range(12, 16):
            for dm in range(2):
                out_group(3, si, dm, tail=True)

    nc.compile()
    return nc


def _get_nc():
    if "nc" not in _CACHE:
        _CACHE["nc"] = _build_nc()
    return _CACHE["nc"]


def make_in_maps(x, mask, Wq, bq, Wk, bk, Wv, bv, Wo, bo):
    import ml_dtypes
    f32 = np.float32
    bf16 = ml_dtypes.bfloat16
    # trineg[kc, qc] = -1e30 where qc < kc (query before key -> masked)
    tri = np.triu(np.ones((128, 128), f32)).astype(bf16)
    trimask2 = np.concatenate([tri, tri], axis=1)
    in_maps = []
    for c in range(NCORES):
        b, g = c // 2, c % 2
        xb = np.asarray(x[b], f32)  # [S, D]
        xw = np.ascontiguousarray(
            xb.reshape(NJ1, SC1, 8, 128).transpose(0, 3, 2, 1).reshape(
                NJ1, 128, 8 * SC1)).astype(bf16)
        sl = slice(g * 512, (g + 1) * 512)

        def wlay(W):  # [512,1024] rows=outputs -> [128, 8*512]
            return np.ascontiguousarray(
                np.asarray(W[sl], f32).reshape(512, 8, 128).transpose(2, 1, 0)
                .reshape(128, 4096)).astype(bf16)

        wo = np.ascontiguousarray(
            np.asarray(Wo[:, sl], f32).T.reshape(4, 128, 1024)
            .transpose(1, 0, 2).reshape(128, 4096)).astype(bf16)
        bq2 = np.ascontiguousarray(np.asarray(bq[sl], f32).reshape(4, 128).T)
        bk2 = np.ascontiguousarray(np.asarray(bk[sl], f32).reshape(4, 128).T)
        bvrow = np.asarray(bv[sl], f32).reshape(1, 512).astype(bf16)
        kbias = np.ascontiguousarray(
            np.where(np.asarray(mask[b]) == 0, f32(-1e30), f32(0.0))
            .astype(f32).reshape(NKT, 128).T)
        in_maps.append({
            "xw": xw, "wq": wlay(Wq), "wk": wlay(Wk), "wv": wlay(Wv),
            "wo": wo, "bq2": bq2, "bk2": bk2, "bvrow": bvrow,
            "kbias": kbias, "trimask2": trimask2,
        })
    return in_maps


def kernel(x, mask, Wq, bq, Wk, bk, Wv, bv, Wo, bo):
    from concourse.bass_utils import run_bass_kernel_spmd

    nc = _get_nc()
    in_maps = make_in_maps(x, mask, Wq, bq, Wk, bk, Wv, bv, Wo, bo)
    res = run_bass_kernel_spmd(nc, in_maps, list(range(NCORES))).results
    out = np.empty((B, S, D), np.float32)
    bo32 = np.asarray(bo, np.float32)
    for b in range(B):
        out[b] = res[2 * b]["out"] + res[2 * b + 1]["out"] + bo32
    return out
